# revision 30
# baseline (speedup 1.0000x reference)
"""Multi-head attention (B=4, N=2048, C=768, H=12) on 8 TRN2 NeuronCores.

Sharding: 4 batches x 2 head-groups (6 heads each); core = 2*b + g.
Attention/qkv are head-sharded exactly as before; the OUTPUT PROJECTION is
sharded by query ROWS instead of columns, with a per-core chunk
permutation so every tail dependency is local:

  - host rolls x columns by (1-g)*1024, so core g=0 processes logical
    query chunks [2,3,0,1] while g=1 processes [0,1,2,3] (attention over
    the full rolled key set is permutation-invariant).
  - core g outputs y rows of its OWN logical half: those come from its
    LAST two program chunks (own heads, straight from SBUF) plus the
    partner's heads for the same logical rows - which the partner
    computed in its FIRST two program chunks. So only program chunks 0,1
    are AllGathered, both early and fully hidden; chunks 2,3 need no
    collective and the kernel tail is a handful of local matmuls.
  - rank-asymmetry of the gathered buffer (own half at rank-dependent
    rows) is neutralized host-side: the gathered-path weight tensor has
    the core's own 384 w_out rows zeroed, and the own-heads contraction
    runs separately against a per-core w_out row slice. (6 gathered + 3
    local contraction steps per y block, +18K PE cycles total.)

Per core attention (unchanged): flash-style on transposed-S tiles:
  S^T = kT.T @ qT (PE, bf16, head pair in disjoint row groups),
  P^T = exp(S^T/8) (ACT), o^T = [v|1].T @ P^T (PE; ones column
  accumulates softmax row-sums), normalize via reciprocal + K=1
  broadcast matmul.

Scheduling: chunk-major, head-pair-rotating; qkv + projection work cut
into single-PSUM-group units drained one per attention mj-iteration.
y rows 512-1023 (program chunk 3) use partial-accumulation: 8-matmul
partials (gathered + hp0 + hp1 + bias) land in SBUF during the last
attention chunk; after the final norm only 8 single-matmul finishes +
adds remain. Host concatenates row halves (no host math).
"""

import sys

sys.path.insert(0, "/opt/trn_rl_repo")

import ml_dtypes
import numpy as np

import concourse.bass as bass
import concourse.mybir as mybir
from concourse import bacc, tile
from concourse.bass_utils import run_bass_kernel_spmd

F32 = mybir.dt.float32
BF16 = mybir.dt.bfloat16

B, N, C, H, D = 4, 2048, 768, 12, 64
G = 2               # head groups (tensor-parallel dim)
HPC = H // G        # heads per core = 6
KC = HPC * D        # per-core head width = 384
CT = C // 128       # contraction tiles over C = 6
NT = N // 128       # 128-row seq tiles = 16
HALF = N // 2       # y rows per core = 1024
SCALE = D ** -0.5


def _build():
    nc = bacc.Bacc(None, num_devices=8)

    # all inputs host-packed to [128, ...] sbuf layout so every DMA is 128
    # contiguous rows (descriptor-row count, not bytes, bounds small DMAs)
    xT_d = nc.declare_dram_parameter("xT", [128, 4 * CT * 512], BF16, isOutput=False)
    wq_d = nc.declare_dram_parameter("wq", [128, CT * KC], BF16, isOutput=False)
    wk_d = nc.declare_dram_parameter("wk", [128, CT * KC], BF16, isOutput=False)
    wv_d = nc.declare_dram_parameter("wv", [128, CT * KC], BF16, isOutput=False)
    wog_d = nc.declare_dram_parameter("wog", [128, CT * C], BF16, isOutput=False)
    woo_d = nc.declare_dram_parameter("woo", [128, 3 * C], BF16, isOutput=False)
    bb_d = nc.declare_dram_parameter("bb", [128, C], F32, isOutput=False)
    y_d = nc.declare_dram_parameter("y", [HALF, C], BF16, isOutput=True)

    with tile.TileContext(nc) as tc:
        with (
            tc.tile_pool(name="wpool", bufs=1) as wpool,
            tc.tile_pool(name="xpool", bufs=1) as xpool,
            tc.tile_pool(name="seq", bufs=1) as seq,
            tc.tile_pool(name="work", bufs=3) as work,
            tc.tile_pool(name="small", bufs=2) as small,
            tc.tile_pool(name="psum", bufs=2, space="PSUM") as psum,
            tc.tile_pool(name="dram", bufs=1, space="DRAM") as dram,
        ):
            # ---- input DMAs (host supplies bf16, pre-packed) ----
            with nc.named_scope("load"):
                wq_sb = wpool.tile([128, CT, KC], BF16)
                wk_sb = wpool.tile([128, CT, KC], BF16)
                wv_sb = wpool.tile([128, CT, KC], BF16)
                wog_sb = wpool.tile([128, CT, C], BF16)
                woo_sb = wpool.tile([128, 3, C], BF16)
                bb_sb = wpool.tile([128, C], F32)
                # chunk-major x so each 512-col chunk is one contiguous slab
                xT_sb = xpool.tile([128, 4, CT, 512], BF16)
                xT_src = xT_d.rearrange("p (ni r) -> p ni r", ni=4)
                # four in-order queues (~120GB/s each): critical transfers
                # first, split in ct-halves so the earliest matmuls (which
                # consume ct ascending) unblock at half-transfer
                wq_s = wq_d.rearrange("p (ct k) -> p ct k", ct=CT)
                wk_s = wk_d.rearrange("p (ct k) -> p ct k", ct=CT)
                wv_s = wv_d.rearrange("p (ct k) -> p ct k", ct=CT)
                xc = [xT_src[:, i].rearrange("p (ct n) -> p ct n", ct=CT)
                      for i in range(4)]
                nc.sync.dma_start(wq_sb[:, 0:3], wq_s[:, 0:3])
                nc.gpsimd.dma_start(xT_sb[:, 0, 0:3], xc[0][:, 0:3])
                nc.scalar.dma_start(wk_sb[:, 0:3], wk_s[:, 0:3])
                nc.sync.dma_start(wq_sb[:, 3:6], wq_s[:, 3:6])
                nc.gpsimd.dma_start(xT_sb[:, 0, 3:6], xc[0][:, 3:6])
                nc.scalar.dma_start(wk_sb[:, 3:6], wk_s[:, 3:6])
                nc.sync.dma_start(xT_sb[:, 1, 0:3], xc[1][:, 0:3])
                nc.gpsimd.dma_start(xT_sb[:, 2, 0:3], xc[2][:, 0:3])
                nc.scalar.dma_start(wv_sb[:, 0:3], wv_s[:, 0:3])
                nc.sync.dma_start(xT_sb[:, 1, 3:6], xc[1][:, 3:6])
                nc.gpsimd.dma_start(xT_sb[:, 2, 3:6], xc[2][:, 3:6])
                nc.scalar.dma_start(wv_sb[:, 3:6], wv_s[:, 3:6])
                nc.scalar.dma_start(xT_sb[:, 3, 0:3], xc[3][:, 0:3])
                nc.scalar.dma_start(xT_sb[:, 3, 3:6], xc[3][:, 3:6])
                nc.gpsimd.dma_start(wog_sb[:], wog_d.rearrange("p (ct k) -> p ct k", ct=CT))
                nc.sync.dma_start(woo_sb[:], woo_d.rearrange("p (t k) -> p t k", t=3))
                nc.sync.dma_start(bb_sb[:], bb_d[:])

            # ---- persistent tiles ----
            qT_sb = [seq.tile([128, N], BF16, name=f"qT{t}", tag=f"qT{t}") for t in range(3)]
            kT_sb = [seq.tile([128, N], BF16, name=f"kT{t}", tag=f"kT{t}") for t in range(3)]
            v_sb = seq.tile([128, NT * HPC * 65], BF16, tag="v")
            # ones column at offset 64 of every 65-wide block (softmax row-sum trick)
            nc.vector.memset(v_sb.rearrange("p (b s) -> p b s", s=65)[:, :, 64], 1.0)
            ao_sb = [seq.tile([128, N], BF16, name=f"ao{t}", tag=f"ao{t}") for t in range(3)]
            ones_sb = small.tile([1, 64], BF16, bufs=1)
            nc.vector.memset(ones_sb[:], 1.0)
            # AllGather bounce buffers for program chunks 0 and 1 only
            ag_in = [dram.tile([KC, 512], BF16, name=f"ag_in{i}") for i in range(2)]
            ag_out = [dram.tile([C, 512], BF16, name=f"ag_out{i}") for i in range(2)]

            # ---- background work units (one PSUM group each) ----
            def qk_group(wsb, dst, hp, ni):
                # qT or kT for head-pair hp, columns ni*512:(ni+1)*512
                with nc.named_scope("qkv"):
                    qk_ps = psum.tile([128, 512], F32, name="qk_ps", tag="mm")
                    for ct in range(CT):
                        nc.tensor.matmul(
                            qk_ps[:],
                            wsb[:, ct, hp * 128:(hp + 1) * 128],
                            xT_sb[:, ni, ct, :],
                            start=(ct == 0), stop=(ct == CT - 1),
                        )
                    nc.vector.tensor_copy(dst[:, ni * 512:(ni + 1) * 512], qk_ps[:])

            def v_group(mj):
                with nc.named_scope("qkv"):
                    v_ps = psum.tile([128, KC], F32, name="v_ps", tag="mm")
                    for ct in range(CT):
                        nc.tensor.matmul(
                            v_ps[:],
                            xT_sb[:, mj // 4, ct, (mj % 4) * 128:(mj % 4) * 128 + 128],
                            wv_sb[:, ct, :],
                            start=(ct == 0), stop=(ct == CT - 1),
                        )
                    base = mj * HPC * 65
                    dst = v_sb[:, base:base + HPC * 65]
                    dst = dst.rearrange("p (h s) -> p h s", s=65)[:, :, 0:64]
                    nc.vector.tensor_copy(
                        dst, v_ps.rearrange("p (h d) -> p h d", d=64)
                    )

            aoF = {}  # hc -> sbuf tile holding gathered partner chunk

            def proj_load(hc):
                # one strided DMA for the gathered chunk, from the gpsimd
                # sequencer to dodge the busy sync queue
                with nc.named_scope(f"projld{hc}"):
                    t = work.tile([128, CT, 512], BF16, name=f"aoF{hc}",
                                  tag="aoF", bufs=2)
                    src = ag_out[hc].rearrange("(kt p) n -> p kt n", p=128)
                    nc.gpsimd.dma_start(t[:, 0:3, :], src[:, 0:3, :])
                    nc.gpsimd.dma_start(t[:, 3:CT, :], src[:, 3:CT, :])
                    aoF[hc] = t

            part_sb = {}

            def proj_partial(hc, nj, s):
                # gathered-path partial for y rows hc*512+nj*128, cols s*384:
                # 6 matmuls against the gathered chunk (own rows zeroed in
                # wog) + bias, parked in SBUF. Ready as soon as the gather
                # lands - this is the mid-kernel PE filler.
                with nc.named_scope(f"proj{hc}p"):
                    y_ps = psum.tile([128, KC], F32, name="yp_ps", tag="mm")
                    for j in range(CT):
                        nc.tensor.matmul(
                            y_ps[:],
                            aoF[hc][:, j, nj * 128:(nj + 1) * 128],
                            wog_sb[:, j, s * KC:(s + 1) * KC],
                            start=(j == 0), stop=(j == CT - 1),
                        )
                    p = work.tile([128, KC], F32, name=f"part{hc}_{nj}_{s}",
                                  tag=f"part{hc}_{nj}_{s}", bufs=1)
                    nc.vector.tensor_add(p[:], y_ps[:], bb_sb[:, s * KC:(s + 1) * KC])
                    part_sb[(hc, nj, s)] = p

            def proj_finish(hc, nj, s):
                # 3 local contraction steps (own heads, straight from SBUF)
                # + the parked partial
                with nc.named_scope(f"proj{hc}f"):
                    y_ps = psum.tile([128, KC], F32, name="yf_ps", tag="mm")
                    col = (hc + 2) * 512 + nj * 128
                    for t in range(3):
                        nc.tensor.matmul(
                            y_ps[:],
                            ao_sb[t][:, col:col + 128],
                            woo_sb[:, t, s * KC:(s + 1) * KC],
                            start=(t == 0), stop=(t == 2),
                        )
                    y_sb = work.tile([128, KC], BF16, name="y_sb", tag="y")
                    nc.vector.tensor_add(y_sb[:], y_ps[:], part_sb[(hc, nj, s)][:])
                    nc.gpsimd.dma_start(
                        y_d[hc * 512 + nj * 128: hc * 512 + (nj + 1) * 128,
                            s * KC:(s + 1) * KC], y_sb[:]
                    )

            from collections import deque
            bg = deque()

            def drain_bg(n=1):
                for _ in range(n):
                    if bg:
                        bg.popleft()()

            r_rows = {}  # (hp, c, i) -> stashed softmax row-sum row

            def attn_chunk(hp, c, defer_norm=True):
                # attention for head-pair hp over query columns c*512:(c+1)*512.
                # The norm block is deferred into the NEXT chunk's bg queue
                # (stashes first, freeing the oT banks).
                col = c * 512
                with nc.named_scope(f"attn{c}"):
                    t = hp
                    kT_h, qT_h = kT_sb[t], qT_sb[t]
                    oT = [
                        psum.tile([65, 512], F32, name=f"oT{i}", tag="oT")
                        for i in range(2)
                    ]
                    # spread the chunk's background allotment evenly over the
                    # 16 mj iterations: draining 1/mj unconditionally empties
                    # a short queue early and leaves the chunk tail ACT-paced
                    k = len(bg)
                    drains = [0] * NT
                    for j in range(min(k, NT)):
                        drains[(j * NT) // min(k, NT) if k < NT else j] += 1
                    for mp in range(NT // 2):
                        # two key-tiles' S^T into one 4-bank psum tile so a
                        # single 2048-wide ACT covers both: halves the ACT
                        # per-instruction overhead (~185ns) so thin chunks
                        # stop being exp-paced. Single-buffered: the next
                        # pair's S waits on this exp read, which PV + bg work
                        # covers.
                        drain_bg(drains[2 * mp] + drains[2 * mp + 1])
                        sT = psum.tile([128, 2048], F32, name="sT", tag="sT", bufs=1)
                        for par in range(2):
                            mj = 2 * mp + par
                            for i in range(2):  # head within pair, row group i*64
                                po = i * 64
                                nc.tensor.matmul(
                                    sT[:, par * 1024 + i * 512:
                                       par * 1024 + (i + 1) * 512],
                                    kT_h[po:po + 64, mj * 128:(mj + 1) * 128],
                                    qT_h[po:po + 64, col:col + 512],
                                    start=True, stop=True,
                                )
                        pT = work.tile([128, 2048], BF16, name="pT", tag="pT", bufs=4)
                        nc.scalar.activation(
                            pT[:], sT[:], mybir.ActivationFunctionType.Exp, scale=SCALE,
                        )
                        for par in range(2):
                            mj = 2 * mp + par
                            for i in range(2):
                                h = hp * 2 + i
                                vblk = v_sb[:, (mj * HPC + h) * 65:(mj * HPC + h) * 65 + 65]
                                nc.tensor.matmul(
                                    oT[i][:], vblk,
                                    pT[:, par * 1024 + i * 512:
                                       par * 1024 + (i + 1) * 512],
                                    start=(mj == 0), stop=(mj == NT - 1),
                                )

                def stash_group():
                    # unnormalized output + row-sums out of PSUM, then the
                    # reciprocal immediately (shortens the later norm chain)
                    with nc.named_scope(f"attn{c}"):
                        for i in range(2):
                            po = i * 64
                            nc.vector.tensor_copy(
                                ao_sb[t][po:po + 64, col:col + 512], oT[i][0:64, :]
                            )
                            r_row = small.tile([1, 512], F32, name="r_row",
                                               tag="r_row", bufs=4)
                            nc.vector.tensor_copy(r_row[:], oT[i][64:65, :])
                            rb_row = small.tile([1, 512], BF16, name="rb_row",
                                                tag="rb_row", bufs=4)
                            rinv = small.tile([1, 512], F32, name="rinv",
                                              tag="rinv", bufs=2)
                            nc.vector.reciprocal_approx_fast(rinv[:], r_row[:])
                            nc.vector.tensor_copy(rb_row[:], rinv[:])
                            r_rows[(t, c, i)] = rb_row

                def norm_group():
                    # K=1 broadcast matmul + in-place scale; ship to the
                    # AllGather staging only for chunks 0 and 1
                    with nc.named_scope(f"attn{c}"):
                        for i in range(2):
                            po = i * 64
                            ao_slice = ao_sb[t][po:po + 64, col:col + 512]
                            rb_ps = psum.tile([64, 512], F32, name="rb_ps", tag="mm")
                            nc.tensor.matmul(rb_ps[:], ones_sb[:],
                                             r_rows.pop((t, c, i))[:],
                                             start=True, stop=True)
                            nc.vector.tensor_mul(ao_slice, ao_slice, rb_ps[:])
                            if c < 2:
                                nc.gpsimd.dma_start(
                                    ag_in[c][t * 128 + po: t * 128 + po + 64, :],
                                    ao_slice,
                                )

                if defer_norm:
                    bg.appendleft(norm_group)
                    bg.appendleft(stash_group)
                else:
                    stash_group()
                    norm_group()

            def emit_ag(hc):
                with nc.named_scope(f"ag{hc}"):
                    nc.gpsimd.collective_compute(
                        "AllGather",
                        mybir.AluOpType.bypass,
                        replica_groups=[[0, 1], [2, 3], [4, 5], [6, 7]],
                        ins=[ag_in[hc].opt()],
                        outs=[ag_out[hc].opt()],
                    )

            # ---- emission schedule ----
            # Chunk-major, head-pair-rotating order (as the proven baseline):
            # chunk c completes after its hp=2 pass.
            def qg(hp, ni):
                return lambda: qk_group(wq_sb, qT_sb[hp], hp, ni)

            def kg(hp, ni):
                return lambda: qk_group(wk_sb, kT_sb[hp], hp, ni)

            # minimal prologue: attention starts after x0 + the hp0 k/q
            # groups it immediately needs; the rest of x streams in under
            # the first chunk (kg(0,ni) emitted well before S(4ni) reads it)
            qk_group(wq_sb, qT_sb[0], 0, 0)
            qk_group(wk_sb, kT_sb[0], 0, 0)
            for mj in range(4):
                v_group(mj)

            def vg(mj):
                return lambda: v_group(mj)

            # x2/x3-dependent units sit late in the queue so they enter the
            # PE FIFO only after their DMA lands (a too-early unit blocks
            # the whole FIFO); each is still emitted before its reader
            bg.extend([kg(0, 1), vg(4), vg(5), vg(6), vg(7), kg(0, 2),
                       vg(8), vg(9), kg(0, 3), vg(10), vg(11), vg(12),
                       vg(13), vg(14), vg(15)])
            attn_chunk(0, 0)
            # hp1's first k/q inline: they must precede (1,0)'s first S in
            # the PE FIFO, which the spread drain cannot guarantee
            qk_group(wk_sb, kT_sb[1], 1, 0)
            qk_group(wq_sb, qT_sb[1], 1, 0)
            bg.extend([kg(1, 1), kg(1, 2), kg(1, 3), kg(2, 0), kg(2, 1),
                       kg(2, 2), kg(2, 3), qg(2, 0)])
            attn_chunk(1, 0)
            bg.extend([qg(0, 1), qg(1, 1), qg(2, 1)])
            attn_chunk(2, 0)
            bg.extend([qg(0, 2), qg(1, 2), qg(2, 2)])
            attn_chunk(0, 1)
            # chunk 0's last ships were emitted by the deferred norm groups
            # during the chunk above, so the collective may only be emitted now
            emit_ag(0)
            proj_load(0)
            def p0(i):
                nj, s = divmod(i, 2)
                return lambda: proj_partial(0, nj, s)

            def p1(i):
                nj, s = divmod(i, 2)
                return lambda: proj_partial(1, nj, s)

            def f0(i):
                nj, s = divmod(i, 2)
                return lambda: proj_finish(0, nj, s)

            bg.extend([qg(0, 3)])
            attn_chunk(1, 1)
            # gathered-path partials for y rows 0-511 fill the otherwise-dry
            # mid-kernel chunks (only need aoF[0])
            bg.extend([p0(i) for i in range(5)])
            attn_chunk(2, 1)
            bg.extend([p0(i) for i in range(5, 8)])
            attn_chunk(0, 2)
            emit_ag(1)
            proj_load(1)
            bg.extend([qg(1, 3), qg(2, 3)])
            attn_chunk(1, 2)
            bg.extend([p1(i) for i in range(5)])
            attn_chunk(2, 2)
            # finishes for y rows 0-511: unblocked once norm(2,2) drains at
            # the next chunk's start
            bg.extend([p1(i) for i in range(5, 8)] + [f0(i) for i in range(3)])
            attn_chunk(0, 3)
            bg.extend([f0(i) for i in range(3, 6)])
            attn_chunk(1, 3)
            bg.extend([f0(i) for i in range(6, 8)])
            attn_chunk(2, 3, defer_norm=False)
            drain_bg(len(bg))
            for nj in range(4):
                for s in range(2):
                    proj_finish(1, nj, s)

    nc.finalize()
    return nc


_NC = None
LAST_RESULTS = None


def _get_nc():
    global _NC
    if _NC is None:
        _NC = _build()
    return _NC


def kernel(x, w_qkv, w_out, b_out, _trace=False):
    global LAST_RESULTS
    nc = _get_nc()

    x = np.asarray(x, dtype=np.float32)
    w_qkv = np.asarray(w_qkv, dtype=np.float32)
    w_out = np.asarray(w_out, dtype=np.float32)
    b_out = np.asarray(b_out, dtype=np.float32)

    bf16 = ml_dtypes.bfloat16
    bb = np.tile(b_out, (128, 1))

    def pack(w, groups):  # [groups*128, k] -> [128, groups*k] sbuf layout
        k = w.shape[1]
        return np.ascontiguousarray(
            w.reshape(groups, 128, k).transpose(1, 0, 2).reshape(128, -1)
        ).astype(bf16)

    in_maps = []
    for cid in range(8):
        b, g = cid // 2, cid % 2
        s = g * KC
        # roll so program chunks 2,3 are this core's own logical rows
        # (g=0 -> logical chunks [2,3,0,1], g=1 -> [0,1,2,3])
        xT = np.roll(x[b].T, -(1 - g) * HALF, axis=1)
        # chunk-major pack: [128, ni, ct, 512]
        xTp = np.ascontiguousarray(
            xT.reshape(CT, 128, 4, 512).transpose(1, 2, 0, 3).reshape(128, -1)
        ).astype(bf16)
        wog = w_out.copy()
        wog[s:s + KC, :] = 0.0  # own rows come via the local path instead
        in_maps.append({
            "xT": xTp,
            "wq": pack(w_qkv[:, s:s + KC], CT),
            "wk": pack(w_qkv[:, C + s:C + s + KC], CT),
            "wv": pack(w_qkv[:, 2 * C + s:2 * C + s + KC], CT),
            "wog": pack(wog, CT),
            "woo": pack(w_out[s:s + KC, :], 3),
            "bb": bb,
        })

    res = run_bass_kernel_spmd(nc, in_maps, core_ids=list(range(8)), trace=_trace)
    LAST_RESULTS = res

    out = np.empty((B, N, C), dtype=np.float32)
    for cid in range(8):
        b, g = cid // 2, cid % 2
        # core g's y rows are its logical half: program chunks 2,3 map to
        # logical chunks (2+ (1-g)*2) mod 4 .. = rows g*1024..g*1024+1023
        out[b, g * HALF:(g + 1) * HALF, :] = res.results[cid]["y"].astype(np.float32)
    return out


# revision 31
# speedup vs baseline: 1.5475x; 1.5475x over previous
"""Multi-head attention (B=4, N=2048, C=768, H=12) on 8 TRN2 NeuronCores.

Sharding: 4 batches x 2 head-groups (6 heads each); core = 2*b + g.
Attention/qkv are head-sharded exactly as before; the OUTPUT PROJECTION is
sharded by query ROWS instead of columns, with a per-core chunk
permutation so every tail dependency is local:

  - host rolls x columns by (1-g)*1024, so core g=0 processes logical
    query chunks [2,3,0,1] while g=1 processes [0,1,2,3] (attention over
    the full rolled key set is permutation-invariant).
  - core g outputs y rows of its OWN logical half: those come from its
    LAST two program chunks (own heads, straight from SBUF) plus the
    partner's heads for the same logical rows - which the partner
    computed in its FIRST two program chunks. So only program chunks 0,1
    are AllGathered, both early and fully hidden; chunks 2,3 need no
    collective and the kernel tail is a handful of local matmuls.
  - rank-asymmetry of the gathered buffer (own half at rank-dependent
    rows) is neutralized host-side: the gathered-path weight tensor has
    the core's own 384 w_out rows zeroed, and the own-heads contraction
    runs separately against a per-core w_out row slice. (6 gathered + 3
    local contraction steps per y block, +18K PE cycles total.)

Per core attention (unchanged): flash-style on transposed-S tiles:
  S^T = kT.T @ qT (PE, bf16, head pair in disjoint row groups),
  P^T = exp(S^T/8) (ACT), o^T = [v|1].T @ P^T (PE; ones column
  accumulates softmax row-sums), normalize via reciprocal + K=1
  broadcast matmul.

Scheduling: chunk-major, head-pair-rotating; qkv + projection work cut
into single-PSUM-group units drained one per attention mj-iteration.
y rows 512-1023 (program chunk 3) use partial-accumulation: 8-matmul
partials (gathered + hp0 + hp1 + bias) land in SBUF during the last
attention chunk; after the final norm only 8 single-matmul finishes +
adds remain. Host concatenates row halves (no host math).
"""

import sys

sys.path.insert(0, "/opt/trn_rl_repo")

import ml_dtypes
import numpy as np

import concourse.bass as bass
import concourse.mybir as mybir
from concourse import bacc, tile
from concourse.bass_utils import run_bass_kernel_spmd

F32 = mybir.dt.float32
BF16 = mybir.dt.bfloat16

B, N, C, H, D = 4, 2048, 768, 12, 64
G = 2               # head groups (tensor-parallel dim)
HPC = H // G        # heads per core = 6
KC = HPC * D        # per-core head width = 384
CT = C // 128       # contraction tiles over C = 6
NT = N // 128       # 128-row seq tiles = 16
HALF = N // 2       # y rows per core = 1024
SCALE = D ** -0.5


def _build():
    nc = bacc.Bacc(None, num_devices=8)

    # all inputs host-packed to [128, ...] sbuf layout so every DMA is 128
    # contiguous rows (descriptor-row count, not bytes, bounds small DMAs)
    xT_d = nc.declare_dram_parameter("xT", [128, 4 * CT * 512], BF16, isOutput=False)
    wq_d = nc.declare_dram_parameter("wq", [128, CT * KC], BF16, isOutput=False)
    wk_d = nc.declare_dram_parameter("wk", [128, CT * KC], BF16, isOutput=False)
    wv_d = nc.declare_dram_parameter("wv", [128, CT * KC], BF16, isOutput=False)
    wog_d = nc.declare_dram_parameter("wog", [128, CT * C], BF16, isOutput=False)
    woo_d = nc.declare_dram_parameter("woo", [128, 3 * C], BF16, isOutput=False)
    bb_d = nc.declare_dram_parameter("bb", [128, C], F32, isOutput=False)
    y_d = nc.declare_dram_parameter("y", [HALF, C], BF16, isOutput=True)

    with tile.TileContext(nc) as tc:
        with (
            tc.tile_pool(name="wpool", bufs=1) as wpool,
            tc.tile_pool(name="xpool", bufs=1) as xpool,
            tc.tile_pool(name="seq", bufs=1) as seq,
            tc.tile_pool(name="work", bufs=3) as work,
            tc.tile_pool(name="small", bufs=2) as small,
            tc.tile_pool(name="psum", bufs=2, space="PSUM") as psum,
            tc.tile_pool(name="dram", bufs=1, space="DRAM") as dram,
        ):
            # ---- input DMAs (host supplies bf16, pre-packed) ----
            with nc.named_scope("load"):
                wq_sb = wpool.tile([128, CT, KC], BF16)
                wk_sb = wpool.tile([128, CT, KC], BF16)
                wv_sb = wpool.tile([128, CT, KC], BF16)
                wog_sb = wpool.tile([128, CT, C], BF16)
                woo_sb = wpool.tile([128, 3, C], BF16)
                bb_sb = wpool.tile([128, C], F32)
                # chunk-major x so each 512-col chunk is one contiguous slab
                xT_sb = xpool.tile([128, 4, CT, 512], BF16)
                xT_src = xT_d.rearrange("p (ni r) -> p ni r", ni=4)
                # four in-order queues (~120GB/s each): critical transfers
                # first, split in ct-halves so the earliest matmuls (which
                # consume ct ascending) unblock at half-transfer
                wq_s = wq_d.rearrange("p (ct k) -> p ct k", ct=CT)
                wk_s = wk_d.rearrange("p (ct k) -> p ct k", ct=CT)
                wv_s = wv_d.rearrange("p (ct k) -> p ct k", ct=CT)
                xc = [xT_src[:, i].rearrange("p (ct n) -> p ct n", ct=CT)
                      for i in range(4)]
                nc.sync.dma_start(wq_sb[:, 0:3], wq_s[:, 0:3])
                nc.gpsimd.dma_start(xT_sb[:, 0, 0:3], xc[0][:, 0:3])
                nc.scalar.dma_start(wk_sb[:, 0:3], wk_s[:, 0:3])
                nc.sync.dma_start(wq_sb[:, 3:6], wq_s[:, 3:6])
                nc.gpsimd.dma_start(xT_sb[:, 0, 3:6], xc[0][:, 3:6])
                nc.scalar.dma_start(wk_sb[:, 3:6], wk_s[:, 3:6])
                nc.sync.dma_start(xT_sb[:, 1, 0:3], xc[1][:, 0:3])
                nc.gpsimd.dma_start(xT_sb[:, 2, 0:3], xc[2][:, 0:3])
                nc.scalar.dma_start(wv_sb[:, 0:3], wv_s[:, 0:3])
                nc.sync.dma_start(xT_sb[:, 1, 3:6], xc[1][:, 3:6])
                nc.gpsimd.dma_start(xT_sb[:, 2, 3:6], xc[2][:, 3:6])
                nc.scalar.dma_start(wv_sb[:, 3:6], wv_s[:, 3:6])
                nc.scalar.dma_start(xT_sb[:, 3, 0:3], xc[3][:, 0:3])
                nc.scalar.dma_start(xT_sb[:, 3, 3:6], xc[3][:, 3:6])
                nc.gpsimd.dma_start(wog_sb[:], wog_d.rearrange("p (ct k) -> p ct k", ct=CT))
                nc.sync.dma_start(woo_sb[:], woo_d.rearrange("p (t k) -> p t k", t=3))
                nc.sync.dma_start(bb_sb[:], bb_d[:])

            # ---- persistent tiles ----
            qT_sb = [seq.tile([128, N], BF16, name=f"qT{t}", tag=f"qT{t}") for t in range(3)]
            kT_sb = [seq.tile([128, N], BF16, name=f"kT{t}", tag=f"kT{t}") for t in range(3)]
            v_sb = seq.tile([128, NT * HPC * 65], BF16, tag="v")
            # ones column at offset 64 of every 65-wide block (softmax row-sum trick)
            nc.vector.memset(v_sb.rearrange("p (b s) -> p b s", s=65)[:, :, 64], 1.0)
            ao_sb = [seq.tile([128, N], BF16, name=f"ao{t}", tag=f"ao{t}") for t in range(3)]
            ones_sb = small.tile([1, 64], BF16, bufs=1)
            nc.vector.memset(ones_sb[:], 1.0)
            # AllGather bounce buffers for program chunks 0 and 1 only
            ag_in = [dram.tile([KC, 512], BF16, name=f"ag_in{i}") for i in range(2)]
            ag_out = [dram.tile([C, 512], BF16, name=f"ag_out{i}") for i in range(2)]

            # ---- background work units (one PSUM group each) ----
            def qk_group(wsb, dst, hp, ni):
                # qT or kT for head-pair hp, columns ni*512:(ni+1)*512
                with nc.named_scope("qkv"):
                    qk_ps = psum.tile([128, 512], F32, name="qk_ps", tag="mm")
                    for ct in range(CT):
                        nc.tensor.matmul(
                            qk_ps[:],
                            wsb[:, ct, hp * 128:(hp + 1) * 128],
                            xT_sb[:, ni, ct, :],
                            start=(ct == 0), stop=(ct == CT - 1),
                        )
                    nc.vector.tensor_copy(dst[:, ni * 512:(ni + 1) * 512], qk_ps[:])

            def v_group(mj):
                with nc.named_scope("qkv"):
                    v_ps = psum.tile([128, KC], F32, name="v_ps", tag="mm")
                    for ct in range(CT):
                        nc.tensor.matmul(
                            v_ps[:],
                            xT_sb[:, mj // 4, ct, (mj % 4) * 128:(mj % 4) * 128 + 128],
                            wv_sb[:, ct, :],
                            start=(ct == 0), stop=(ct == CT - 1),
                        )
                    base = mj * HPC * 65
                    dst = v_sb[:, base:base + HPC * 65]
                    dst = dst.rearrange("p (h s) -> p h s", s=65)[:, :, 0:64]
                    nc.vector.tensor_copy(
                        dst, v_ps.rearrange("p (h d) -> p h d", d=64)
                    )

            aoF = {}  # hc -> sbuf tile holding gathered partner chunk

            def proj_load(hc):
                # one strided DMA for the gathered chunk, from the gpsimd
                # sequencer to dodge the busy sync queue
                with nc.named_scope(f"projld{hc}"):
                    t = work.tile([128, CT, 512], BF16, name=f"aoF{hc}",
                                  tag="aoF", bufs=2)
                    src = ag_out[hc].rearrange("(kt p) n -> p kt n", p=128)
                    nc.gpsimd.dma_start(t[:, 0:3, :], src[:, 0:3, :])
                    nc.gpsimd.dma_start(t[:, 3:CT, :], src[:, 3:CT, :])
                    aoF[hc] = t

            part_sb = {}

            def proj_partial(hc, nj, s):
                # gathered-path partial for y rows hc*512+nj*128, cols s*384:
                # 6 matmuls against the gathered chunk (own rows zeroed in
                # wog) + bias, parked in SBUF. Ready as soon as the gather
                # lands - this is the mid-kernel PE filler.
                with nc.named_scope(f"proj{hc}p"):
                    y_ps = psum.tile([128, KC], F32, name="yp_ps", tag="mm")
                    for j in range(CT):
                        nc.tensor.matmul(
                            y_ps[:],
                            aoF[hc][:, j, nj * 128:(nj + 1) * 128],
                            wog_sb[:, j, s * KC:(s + 1) * KC],
                            start=(j == 0), stop=(j == CT - 1),
                        )
                    p = work.tile([128, KC], F32, name=f"part{hc}_{nj}_{s}",
                                  tag=f"part{hc}_{nj}_{s}", bufs=1)
                    nc.vector.tensor_add(p[:], y_ps[:], bb_sb[:, s * KC:(s + 1) * KC])
                    part_sb[(hc, nj, s)] = p

            def proj_finish(hc, nj, s):
                # 3 local contraction steps (own heads, straight from SBUF)
                # + the parked partial
                with nc.named_scope(f"proj{hc}f"):
                    y_ps = psum.tile([128, KC], F32, name="yf_ps", tag="mm")
                    col = (hc + 2) * 512 + nj * 128
                    for t in range(3):
                        nc.tensor.matmul(
                            y_ps[:],
                            ao_sb[t][:, col:col + 128],
                            woo_sb[:, t, s * KC:(s + 1) * KC],
                            start=(t == 0), stop=(t == 2),
                        )
                    y_sb = work.tile([128, KC], BF16, name="y_sb", tag="y")
                    nc.vector.tensor_add(y_sb[:], y_ps[:], part_sb[(hc, nj, s)][:])
                    nc.gpsimd.dma_start(
                        y_d[hc * 512 + nj * 128: hc * 512 + (nj + 1) * 128,
                            s * KC:(s + 1) * KC], y_sb[:]
                    )

            from collections import deque
            bg = deque()

            def drain_bg(n=1):
                for _ in range(n):
                    if bg:
                        bg.popleft()()

            r_rows = {}  # (hp, c, i) -> stashed softmax row-sum row

            def attn_chunk(hp, c, defer_norm=True):
                # attention for head-pair hp over query columns c*512:(c+1)*512.
                # The norm block is deferred into the NEXT chunk's bg queue
                # (stashes first, freeing the oT banks).
                col = c * 512
                with nc.named_scope(f"attn{c}"):
                    t = hp
                    kT_h, qT_h = kT_sb[t], qT_sb[t]
                    oT = [
                        psum.tile([65, 512], F32, name=f"oT{i}", tag="oT")
                        for i in range(2)
                    ]
                    # spread the chunk's background allotment evenly over the
                    # 16 mj iterations: draining 1/mj unconditionally empties
                    # a short queue early and leaves the chunk tail ACT-paced
                    k = len(bg)
                    drains = [0] * NT
                    for j in range(min(k, NT)):
                        drains[(j * NT) // min(k, NT) if k < NT else j] += 1
                    for mj in range(NT):
                        drain_bg(drains[mj])
                        # both heads' S^T tiles share one 2-bank psum tile so a
                        # single 1024-wide ACT covers both heads' exp
                        sT = psum.tile([128, 1024], F32, name="sT", tag="sT", bufs=2)
                        for i in range(2):  # i = head within pair, PE row group i*64
                            po = i * 64
                            nc.tensor.matmul(
                                sT[:, i * 512:(i + 1) * 512],
                                kT_h[po:po + 64, mj * 128:(mj + 1) * 128],
                                qT_h[po:po + 64, col:col + 512],
                                start=True, stop=True,
                            )
                        pT = work.tile([128, 1024], BF16, name="pT", tag="pT", bufs=8)
                        nc.scalar.activation(
                            pT[:], sT[:], mybir.ActivationFunctionType.Exp, scale=SCALE,
                        )
                        for i in range(2):
                            h = hp * 2 + i
                            vblk = v_sb[:, (mj * HPC + h) * 65:(mj * HPC + h) * 65 + 65]
                            nc.tensor.matmul(
                                oT[i][:], vblk, pT[:, i * 512:(i + 1) * 512],
                                start=(mj == 0), stop=(mj == NT - 1),
                            )

                def stash_group():
                    # unnormalized output + row-sums out of PSUM, then the
                    # reciprocal immediately (shortens the later norm chain)
                    with nc.named_scope(f"attn{c}"):
                        for i in range(2):
                            po = i * 64
                            nc.vector.tensor_copy(
                                ao_sb[t][po:po + 64, col:col + 512], oT[i][0:64, :]
                            )
                            r_row = small.tile([1, 512], F32, name="r_row",
                                               tag="r_row", bufs=4)
                            nc.vector.tensor_copy(r_row[:], oT[i][64:65, :])
                            rb_row = small.tile([1, 512], BF16, name="rb_row",
                                                tag="rb_row", bufs=4)
                            rinv = small.tile([1, 512], F32, name="rinv",
                                              tag="rinv", bufs=2)
                            nc.vector.reciprocal_approx_fast(rinv[:], r_row[:])
                            nc.vector.tensor_copy(rb_row[:], rinv[:])
                            r_rows[(t, c, i)] = rb_row

                def norm_group():
                    # K=1 broadcast matmul + in-place scale; ship to the
                    # AllGather staging only for chunks 0 and 1
                    with nc.named_scope(f"attn{c}"):
                        for i in range(2):
                            po = i * 64
                            ao_slice = ao_sb[t][po:po + 64, col:col + 512]
                            rb_ps = psum.tile([64, 512], F32, name="rb_ps", tag="mm")
                            nc.tensor.matmul(rb_ps[:], ones_sb[:],
                                             r_rows.pop((t, c, i))[:],
                                             start=True, stop=True)
                            nc.vector.tensor_mul(ao_slice, ao_slice, rb_ps[:])
                            if c < 2:
                                nc.gpsimd.dma_start(
                                    ag_in[c][t * 128 + po: t * 128 + po + 64, :],
                                    ao_slice,
                                )

                if defer_norm:
                    bg.appendleft(norm_group)
                    bg.appendleft(stash_group)
                else:
                    stash_group()
                    norm_group()

            def emit_ag(hc):
                with nc.named_scope(f"ag{hc}"):
                    nc.gpsimd.collective_compute(
                        "AllGather",
                        mybir.AluOpType.bypass,
                        replica_groups=[[0, 1], [2, 3], [4, 5], [6, 7]],
                        ins=[ag_in[hc].opt()],
                        outs=[ag_out[hc].opt()],
                    )

            # ---- emission schedule ----
            # Chunk-major, head-pair-rotating order (as the proven baseline):
            # chunk c completes after its hp=2 pass.
            def qg(hp, ni):
                return lambda: qk_group(wq_sb, qT_sb[hp], hp, ni)

            def kg(hp, ni):
                return lambda: qk_group(wk_sb, kT_sb[hp], hp, ni)

            # minimal prologue: attention starts after x0 + the hp0 k/q
            # groups it immediately needs; the rest of x streams in under
            # the first chunk (kg(0,ni) emitted well before S(4ni) reads it)
            qk_group(wq_sb, qT_sb[0], 0, 0)
            qk_group(wk_sb, kT_sb[0], 0, 0)
            for mj in range(4):
                v_group(mj)

            def vg(mj):
                return lambda: v_group(mj)

            # x2/x3-dependent units sit late in the queue so they enter the
            # PE FIFO only after their DMA lands (a too-early unit blocks
            # the whole FIFO); each is still emitted before its reader
            bg.extend([kg(0, 1), vg(4), vg(5), vg(6), vg(7), kg(0, 2),
                       vg(8), vg(9), kg(0, 3), vg(10), vg(11), vg(12),
                       vg(13), vg(14), vg(15)])
            attn_chunk(0, 0)
            # hp1's first k/q inline: they must precede (1,0)'s first S in
            # the PE FIFO, which the spread drain cannot guarantee
            qk_group(wk_sb, kT_sb[1], 1, 0)
            qk_group(wq_sb, qT_sb[1], 1, 0)
            bg.extend([kg(1, 1), kg(1, 2), kg(1, 3), kg(2, 0), kg(2, 1),
                       kg(2, 2), kg(2, 3), qg(2, 0)])
            attn_chunk(1, 0)
            bg.extend([qg(0, 1), qg(1, 1), qg(2, 1)])
            attn_chunk(2, 0)
            bg.extend([qg(0, 2), qg(1, 2), qg(2, 2)])
            attn_chunk(0, 1)
            # chunk 0's last ships were emitted by the deferred norm groups
            # during the chunk above, so the collective may only be emitted now
            emit_ag(0)
            proj_load(0)
            def p0(i):
                nj, s = divmod(i, 2)
                return lambda: proj_partial(0, nj, s)

            def p1(i):
                nj, s = divmod(i, 2)
                return lambda: proj_partial(1, nj, s)

            def f0(i):
                nj, s = divmod(i, 2)
                return lambda: proj_finish(0, nj, s)

            bg.extend([qg(0, 3)])
            attn_chunk(1, 1)
            # gathered-path partials for y rows 0-511 fill the otherwise-dry
            # mid-kernel chunks (only need aoF[0])
            bg.extend([p0(i) for i in range(5)])
            attn_chunk(2, 1)
            bg.extend([p0(i) for i in range(5, 8)])
            attn_chunk(0, 2)
            emit_ag(1)
            proj_load(1)
            bg.extend([qg(1, 3), qg(2, 3)])
            attn_chunk(1, 2)
            bg.extend([p1(i) for i in range(5)])
            attn_chunk(2, 2)
            # finishes for y rows 0-511: unblocked once norm(2,2) drains at
            # the next chunk's start
            bg.extend([p1(i) for i in range(5, 8)] + [f0(i) for i in range(3)])
            attn_chunk(0, 3)
            bg.extend([f0(i) for i in range(3, 6)])
            attn_chunk(1, 3)
            bg.extend([f0(i) for i in range(6, 8)])
            attn_chunk(2, 3, defer_norm=False)
            drain_bg(len(bg))
            for nj in range(4):
                for s in range(2):
                    proj_finish(1, nj, s)

    nc.finalize()
    return nc


_NC = None
LAST_RESULTS = None


def _get_nc():
    global _NC
    if _NC is None:
        _NC = _build()
    return _NC


def kernel(x, w_qkv, w_out, b_out, _trace=False):
    global LAST_RESULTS
    nc = _get_nc()

    x = np.asarray(x, dtype=np.float32)
    w_qkv = np.asarray(w_qkv, dtype=np.float32)
    w_out = np.asarray(w_out, dtype=np.float32)
    b_out = np.asarray(b_out, dtype=np.float32)

    bf16 = ml_dtypes.bfloat16
    bb = np.tile(b_out, (128, 1))

    def pack(w, groups):  # [groups*128, k] -> [128, groups*k] sbuf layout
        k = w.shape[1]
        return np.ascontiguousarray(
            w.reshape(groups, 128, k).transpose(1, 0, 2).reshape(128, -1)
        ).astype(bf16)

    in_maps = []
    for cid in range(8):
        b, g = cid // 2, cid % 2
        s = g * KC
        # roll so program chunks 2,3 are this core's own logical rows
        # (g=0 -> logical chunks [2,3,0,1], g=1 -> [0,1,2,3])
        xT = np.roll(x[b].T, -(1 - g) * HALF, axis=1)
        # chunk-major pack: [128, ni, ct, 512]
        xTp = np.ascontiguousarray(
            xT.reshape(CT, 128, 4, 512).transpose(1, 2, 0, 3).reshape(128, -1)
        ).astype(bf16)
        wog = w_out.copy()
        wog[s:s + KC, :] = 0.0  # own rows come via the local path instead
        in_maps.append({
            "xT": xTp,
            "wq": pack(w_qkv[:, s:s + KC], CT),
            "wk": pack(w_qkv[:, C + s:C + s + KC], CT),
            "wv": pack(w_qkv[:, 2 * C + s:2 * C + s + KC], CT),
            "wog": pack(wog, CT),
            "woo": pack(w_out[s:s + KC, :], 3),
            "bb": bb,
        })

    res = run_bass_kernel_spmd(nc, in_maps, core_ids=list(range(8)), trace=_trace)
    LAST_RESULTS = res

    out = np.empty((B, N, C), dtype=np.float32)
    for cid in range(8):
        b, g = cid // 2, cid % 2
        # core g's y rows are its logical half: program chunks 2,3 map to
        # logical chunks (2+ (1-g)*2) mod 4 .. = rows g*1024..g*1024+1023
        out[b, g * HALF:(g + 1) * HALF, :] = res.results[cid]["y"].astype(np.float32)
    return out
